# revision 38
# baseline (speedup 1.0000x reference)
"""Trainium2 Bass kernel for nn_AttentionModule.

Computation (per batch row b, input feature i):
    E      = tanh(x @ E_W + E_b)                      # [B, 50]
    s      = einsum('be,iea->bia', E, A_W) + A_b      # [B, 66, 20]
    A      = softmax(s, -1)[..., 1]                   # [B, 66]
    out    = x * A

Key rewrite: softmax(s)[1] = 1 / sum_a exp(s_a - s_1).  We pre-difference the
attention weights on the host (dW = A_W - A_W[:,:,1:2], db likewise), fold the
bias into the matmul via a constant-1 row of E (tanh(0*x + 30) == 1.0), and get

    den[b,i] = sum_a exp(E~ @ W2[:, a*66+i])   (a=1 slab is exactly 0 -> exp=1)
    out[b,i] = x[b,i] / den[b,i]

W2 columns are (i-major, a-minor) so the den groups are contiguous runs
for the DVE reduce (a strided-innermost reduce measured ~2x slower).  The
a=1 column (exactly 0 -> exp 1) is dropped on the host: mm2/exp/reduce all
shrink 5%, and the missing +1 is re-added by a DVE tensor_scalar_add that
(unlike tensor_reduce) runs in the 2x all-SBUF perf mode.  A/B-measured
~3.4% faster end-to-end than carrying the column.

Engine split -- the den pipeline is elementwise-bound (1320 exp + 1320
reduce elems per row at ~1 elem/cyc/lane on every engine), so balance is
everything.  Measured-balanced assignment:
  - PE   : x transposes (fp32; walrus rejects f32r transposes), mm1 (f32r,
           1 cyc/col), mm2 (f32r, 3 PSUM banks)       ~0.68 us/block
  - ACT  : tanh; xT PSUM->SBUF copy; exact exp of all 3 banks in ONE
           strided op [128, 3, 418]                   ~1.52 us/block
  - DVE  : grouped tensor_reduce (TG=4 blocks/op), +1, reciprocal
                                                      ~1.49 us/block
  - Pool : final x*rec multiply (GPSIMD cannot read PSUM, and its
           strided/2-input throughput is far below spec, so it gets only
           this contiguous SBUF op)

Paths explored and rejected on measurement: (a,i) column order + gpsimd
slab-folds (strided reduce cost dominated); Schraudolph bit-trick exp on
DVE (int32(A*s+B) bit pattern, works numerically at ~1.6% rms but only
rebalances toward the busier DVE); hybrid with transposed mm2 chunks +
PE den-matmul against a 0/1 G matrix (verifier-clean and correct at
9.8e-3 but ~2.5x slower -- PSUM bank limits force single-buffered den /
txback tiles and 24 matmuls/macro, serializing the pipeline).
"""

import numpy as np

B_TOTAL, INPUT, E_NODE, A_NODE = 262144, 66, 50, 20
N_CORES = 8
B_LOCAL = B_TOTAL // N_CORES          # 32768
NBLK = 4                              # 128-row blocks per macro tile
MACRO = 128 * NBLK                    # 512
NMACRO = B_LOCAL // MACRO             # 64
A_RED = A_NODE - 1                    # a=1 col dropped: exp(0)=1 re-added
                                      # via a cheap 2x-mode DVE scalar-add
NIA = INPUT * A_RED                   # 1254
CHUNK = NIA // 3                      # 418 cols per PSUM bank
CONST_ROW_BIAS = 30.0                 # tanh(30) == 1.0 in fp32

SCHRAU_A = float(2**23 / np.log(2.0))  # 12102203.16
SCHRAU_B = 1064986822.0

TAIL_GROUP = 4           # blocks per reduce/recip/mul op group
DMA_MACROS = 2           # macros per x-load/y-store DMA
ACT_W = CHUNK             # exact-exp cols per PSUM bank on ACT (of CHUNK=440);
                         # DVE Schraudolph-converts the other 440-ACT_W
XCOPY_ON = "act"         # engine for the xT PSUM->SBUF copy
MM_F32R = True           # f32r mm1 (PE 4x on it)
TX_F32R = False          # f32r transposes (walrus verifier rejects)

_CACHE = {}


def _build_bass(n_rows, repeat=1):
    import concourse.bass as bass
    import concourse.bacc as bacc
    import concourse.tile as tile
    from concourse import mybir
    from concourse.masks import make_identity
    from contextlib import ExitStack

    f32 = mybir.dt.float32
    f32r = mybir.dt.float32r
    i32 = mybir.dt.int32
    nmacro = n_rows // MACRO

    nc = bacc.Bacc("TRN2", target_bir_lowering=False, debug=False,
                   num_devices=N_CORES)

    x_d = nc.dram_tensor("x", [n_rows, INPUT], f32, kind="ExternalInput").ap()
    w1_d = nc.dram_tensor("W1", [INPUT, E_NODE + 1], f32, kind="ExternalInput").ap()
    b1_d = nc.dram_tensor("b1", [E_NODE + 1, 1], f32, kind="ExternalInput").ap()
    w2_d = nc.dram_tensor("W2", [E_NODE + 1, NIA], f32r, kind="ExternalInput").ap()
    y_d = nc.dram_tensor("y", [n_rows, INPUT], f32, kind="ExternalOutput").ap()

    x_r = x_d.rearrange("(m p) f -> m p f", p=128)
    y_r = y_d.rearrange("(m p) f -> m p f", p=128)

    with tile.TileContext(nc) as tc, ExitStack() as ctx:
        const = ctx.enter_context(tc.tile_pool(name="const", bufs=1))
        xpool = ctx.enter_context(tc.tile_pool(name="xp", bufs=3))
        xtp = ctx.enter_context(tc.tile_pool(name="xtp", bufs=2))
        etp = ctx.enter_context(tc.tile_pool(name="etp", bufs=2))
        expp = ctx.enter_context(tc.tile_pool(name="expp", bufs=6))
        denp = ctx.enter_context(tc.tile_pool(name="denp", bufs=6))
        outp = ctx.enter_context(tc.tile_pool(name="outp", bufs=3))
        ps_xt = ctx.enter_context(tc.tile_pool(name="ps_xt", bufs=1, space="PSUM"))
        ps_et = ctx.enter_context(tc.tile_pool(name="ps_et", bufs=1, space="PSUM"))
        ps_s = ctx.enter_context(tc.tile_pool(name="ps_s", bufs=2, space="PSUM"))

        w1_sb = const.tile([INPUT, E_NODE + 1], f32r if MM_F32R else f32)
        nc.sync.dma_start(out=w1_sb, in_=w1_d.bitcast(w1_sb.dtype))
        b1_sb = const.tile([E_NODE + 1, 1], f32)
        nc.sync.dma_start(out=b1_sb, in_=b1_d)
        w2_sb = const.tile([E_NODE + 1, NIA], f32r)
        nc.sync.dma_start(out=w2_sb, in_=w2_d)
        ident = const.tile([128, 128], f32)
        make_identity(nc, ident)
        ident_mm = ident.bitcast(f32r) if MM_F32R else ident
        exp_bias = const.tile([128, 1], f32)
        nc.vector.memset(exp_bias, -SCHRAU_B / SCHRAU_A)

        DM = DMA_MACROS
        assert nmacro % DM == 0
        iters = [m for _ in range(repeat) for m in range(nmacro)]
        xgs = {}

        def emit_load(git):
            """One batched x DMA covering DM consecutive macros."""
            m0 = iters[git * DM]
            xg = xpool.tile([128, DM * NBLK, INPUT], f32)
            nc.sync.dma_start(
                out=xg,
                in_=x_r[m0 * NBLK:m0 * NBLK + DM * NBLK]
                .rearrange("m p f -> p m f"),
            )
            return xg

        def emit_head(it):
            """PE transpose -> copy -> mm1 -> tanh for iteration it."""
            git, off = it // DM, it % DM
            if git not in xgs:
                xgs[git] = emit_load(git)
            x_sb = xgs[git][:, off * NBLK:(off + 1) * NBLK, :]
            xt_ps = ps_xt.tile([INPUT, MACRO], f32)
            for b in range(NBLK):
                src = x_sb[:, b, :]
                dst = xt_ps[:, b * 128:(b + 1) * 128]
                if TX_F32R:
                    src, dst = src.bitcast(f32r), dst.bitcast(f32r)
                nc.tensor.transpose(dst, src,
                                    ident_mm if TX_F32R else ident)
            xt_sb = xtp.tile([INPUT, MACRO], f32r if MM_F32R else f32)
            if XCOPY_ON == "dma":
                nc.sync.dma_start(out=xt_sb, in_=xt_ps.bitcast(xt_sb.dtype))
            elif XCOPY_ON == "act":
                nc.scalar.copy(out=xt_sb, in_=xt_ps)
            else:
                nc.vector.tensor_copy(out=xt_sb, in_=xt_ps)
            et_ps = ps_et.tile([E_NODE + 1, MACRO], f32)
            nc.tensor.matmul(et_ps, w1_sb, xt_sb, start=True, stop=True)
            et_sb = etp.tile([E_NODE + 1, MACRO], f32r)
            nc.scalar.activation(
                et_sb, et_ps, mybir.ActivationFunctionType.Tanh,
                bias=b1_sb, scale=1.0,
            )
            return x_sb, et_sb

        ogs = {}
        heads = {0: emit_head(0)}
        for it in range(len(iters)):
            m = iters[it]
            git, off = it // DM, it % DM
            if it + 1 < len(iters):
                heads[it + 1] = emit_head(it + 1)
            x_sb, et_sb = heads.pop(it)

            if git not in ogs:
                ogs[git] = outp.tile([128, DM * NBLK, INPUT], f32, name="og")
            out_sb = ogs[git][:, off * NBLK:(off + 1) * NBLK, :]
            TG = TAIL_GROUP
            exp_g = None
            for b in range(NBLK):
                bg = b % TG
                # mm2: S [128, NIA] in 3 PSUM banks (cols 0/512/1024)
                s_ps = ps_s.tile([128, 3 * 512], f32)
                lhs = et_sb[:, b * 128:(b + 1) * 128]
                for c in range(3):
                    nc.tensor.matmul(
                        s_ps[:, c * 512:c * 512 + CHUNK], lhs,
                        w2_sb[:, c * CHUNK:(c + 1) * CHUNK],
                        start=True, stop=True,
                    )

                if bg == 0:
                    exp_g = expp.tile([128, TG * NIA], f32, name="exp_g")
                exp_sb = exp_g[:, bg * NIA:(bg + 1) * NIA]

                # mm2 emitted s' = A*s + B (A,B folded into w2 on the host).
                # First ACT_W cols of each bank: exact exp on ACT (scale/
                # bias undo the affine).  Tail of each bank: Schraudolph
                # exp = bits(int32(s')) via one dtype-converting DVE copy.
                # (GPSIMD can't read PSUM, so Pool gets no drain share.)
                s3 = s_ps.rearrange("p (c w) -> p c w", w=512)
                e3 = exp_sb.rearrange("p (c w) -> p c w", w=CHUNK)
                W = ACT_W
                if W < CHUNK:
                    nc.scalar.activation(
                        e3[:, :, 0:W], s3[:, :, 0:W],
                        mybir.ActivationFunctionType.Exp,
                        scale=1.0 / SCHRAU_A, bias=exp_bias,
                    )
                else:
                    nc.scalar.activation(
                        e3[:, :, 0:W], s3[:, :, 0:W],
                        mybir.ActivationFunctionType.Exp,
                    )
                if W < CHUNK:
                    nc.vector.tensor_copy(
                        out=e3[:, :, W:CHUNK].bitcast(i32),
                        in_=s3[:, :, W:CHUNK],
                    )

                if bg != TG - 1:
                    continue

                # grouped den over a: one contiguous-innermost DVE reduce
                # ((i,a) column order -> 20-elem stride-1 groups)
                b0 = b - bg
                g = exp_g.rearrange("p (g a) -> p g a", a=A_RED)
                den = denp.tile([128, TG * INPUT], f32, name="den")
                rec = denp.tile([128, TG * INPUT], f32, name="rec")
                nc.vector.tensor_reduce(
                    out=den, in_=g,
                    axis=mybir.AxisListType.X, op=mybir.AluOpType.add,
                )
                nc.vector.tensor_scalar_add(out=rec, in0=den, scalar1=1.0)
                xs = x_sb[:, b0:b0 + TG, :]
                ys = out_sb[:, b0:b0 + TG, :]
                nc.vector.reciprocal(out=den, in_=rec)
                rec3 = den.rearrange("p (t f) -> p t f", f=INPUT)
                nc.gpsimd.tensor_tensor(
                    out=ys, in0=xs, in1=rec3, op=mybir.AluOpType.mult,
                )

            if off == DM - 1:
                m0 = iters[git * DM]
                og = ogs.pop(git)
                nc.sync.dma_start(
                    out=y_r[m0 * NBLK:m0 * NBLK + DM * NBLK]
                    .rearrange("m p f -> p m f"),
                    in_=og,
                )

    nc.compile()
    return nc


def _prep_weights(E_W, E_b, A_W, A_b):
    E_W = np.asarray(E_W, dtype=np.float32)
    E_b = np.asarray(E_b, dtype=np.float32)
    A_W = np.asarray(A_W, dtype=np.float32)
    A_b = np.asarray(A_b, dtype=np.float32)
    w1 = np.concatenate([E_W, np.zeros((INPUT, 1), np.float32)], axis=1)
    b1 = np.concatenate([E_b, np.float32([CONST_ROW_BIAS])]).reshape(-1, 1)
    dW = A_W - A_W[:, :, 1:2]                        # [66, 50, 20]
    db = A_b - A_b[:, 1:2]                           # [66, 20]
    dW = np.delete(dW, 1, axis=2)                    # [66, 50, 19]
    db = np.delete(db, 1, axis=1)                    # [66, 19]
    # (i-major, a-minor) column order: col = i*20 + a, so den groups are
    # contiguous 20-col runs.  When Schraudolph columns are in play, scale
    # by A and fold B into the bias row so mm2 emits s' = A*s + B directly
    # (f32r truncation of the ~1e9 bias then shifts ALL outputs by a small
    # common factor, which stays well inside the error budget).
    dw2 = dW.transpose(1, 0, 2).reshape(E_NODE, NIA)
    db2 = db.reshape(1, NIA)
    if ACT_W < CHUNK:
        w2 = np.concatenate(
            [dw2 * np.float32(SCHRAU_A),
             db2 * np.float32(SCHRAU_A) + np.float32(SCHRAU_B)], axis=0)
    else:
        w2 = np.concatenate([dw2, db2], axis=0)
    w2 = w2.astype(np.float32)                       # [51, 1320]
    return np.ascontiguousarray(w1), np.ascontiguousarray(b1), \
        np.ascontiguousarray(w2)


def _run(x, E_W, E_b, A_W, A_b, trace=False):
    from concourse.bass_utils import run_bass_kernel_spmd

    x = np.ascontiguousarray(np.asarray(x, dtype=np.float32))
    n_rows_local = x.shape[0] // N_CORES
    key = ("nc", n_rows_local)
    if key not in _CACHE:
        _CACHE[key] = _build_bass(n_rows_local)
    nc = _CACHE[key]

    w1, b1, w2 = _prep_weights(E_W, E_b, A_W, A_b)
    in_maps = [
        {"x": x[i * n_rows_local:(i + 1) * n_rows_local],
         "W1": w1, "b1": b1, "W2": w2}
        for i in range(N_CORES)
    ]
    res = run_bass_kernel_spmd(nc, in_maps, list(range(N_CORES)), trace=trace)
    out = np.concatenate([res.results[i]["y"] for i in range(N_CORES)], axis=0)
    return out, res


def kernel(x, E_W, E_b, A_W, A_b):
    out, _ = _run(x, E_W, E_b, A_W, A_b, trace=False)
    return out
